# revision 10
# baseline (speedup 1.0000x reference)
"""Trainium2 Bass kernel for a (quirky) transformer decoder layer.

Problem shapes: B=2, S=2048, D=128, H=8 heads, head_dim=16.
  sa  = attn(q=x_tgt, kv=x_tgt);  r1 = sa @ w1 + b1 + x_tgt
  ca  = attn(q=enc_out, kv=x_tgt); r2 = ca @ w2 + b2 + r1
  ln  = (r2 - mean) / var   (var unbiased, divide by var not std)
  out = relu(ln @ w3 + b3) @ w4 + b4 + r2
(mask_src / mask_tgt are unused by the reference.)

Sharding: 8 cores, query-row sharding (zero communication). Core c handles
batch c//4, query rows [(c%4)*512 : (c%4+1)*512]. K/V are computed per-core
from the full 2048-row x_tgt of its batch.

v2 design notes (vs the 272us baseline):
- All activations arrive pre-transposed from the host ([d, token]); the
  on-device transpose phase is gone, and the output leaves transposed (host
  untransposes).
- Scores are computed per head with K=16 contraction (lhsT = K_h^T tile,
  rhs = Q_h^T) instead of the K=128 fused-G trick.  K^T/Q^T for 4 heads are
  packed on 32-partition boundaries so score matmuls for 2 heads x 4
  32-key col-tiles co-execute on the PE array via tile_position (the
  128x128 array is 16 independent 32x32 subarrays).
- PV uses col-tiling: each head's [128k,32v] PV matmul targets a 32-col
  slice of one PSUM bank, so 2 heads run concurrently and 4 heads
  accumulate into one bank, which directly produces the packed layout the
  merge projection wants.  The packed V carries a ones-column per head for
  the softmax denominator (as in the baseline).
- exp is split between ScalarE (true Exp activation) and VectorE/GpSimdE
  (Schraudolph bit-trick: round(s*a + b) as int32 IS fp32 exp(s/4) to
  ~2% relative, which softmax normalization mostly cancels).
- reciprocal -> reciprocal_approx_fast (5x).
"""

import numpy as np

import concourse.bass as bass
import concourse.tile as tile
from concourse import mybir
from concourse.bass_utils import run_bass_kernel_spmd

B, S, D, H, HD = 2, 2048, 128, 8, 16
QC = 512  # query rows per core
NCORES = 8
KT = 16  # number of 128-row key tiles
F32 = mybir.dt.float32
F32R = mybir.dt.float32r
I16 = mybir.dt.int16
BF16 = mybir.dt.bfloat16
AF = mybir.ActivationFunctionType
OP = mybir.AluOpType

# Schraudolph exp(s/4) constants: int16 bits of bf16 = s * (0.25 * 2^7/ln2) + B
SCHRA_A = 0.25 * (2.0**7) / np.log(2.0)
SCHRA_B = 127.0 * 2.0**7 - 0.0579 * 2.0**7

# exp engine schedule per k-tile (16 entries): S=ScalarE, V=VectorE, P=GpSimd
EXP_SCHED = "SSVSSVSSVSSVSSVS"


# ---------------------------------------------------------------- host packing
def _wk_quad(w, g):
    """[128, 128] lhsT for the K^T projection of head quad g: out partition
    32i+j (i<4, j<16) = K^T row j of head 4g+i, i.e. col 32i+j = w[:, j*H + 4g+i]."""
    out = np.zeros((D, 128), np.float32)
    for i in range(4):
        for j in range(HD):
            out[:, 32 * i + j] = w[:, j * H + 4 * g + i]
    return out


def _pack32_cols(w, grp):
    """[D, 128]: col 32g+j (j<16) = w[:, j*H + (4*grp+g)], else 0 (V packing;
    col 32g+16 stays 0 and is later set to 1 for the softmax denominator)."""
    out = np.zeros((D, 128), np.float32)
    for g in range(4):
        h = 4 * grp + g
        for j in range(HD):
            out[:, 32 * g + j] = w[:, j * H + h]
    return out


def _pack_w12(w, grp):
    """lhsT for the merge projection: row 32c+j = w[j*H + (4*grp+c), :]."""
    out = np.zeros((D, D), np.float32)
    for c in range(4):
        h = 4 * grp + c
        for j in range(HD):
            out[32 * c + j, :] = w[j * H + h, :]
    return out


def _sel_matrix():
    sel = np.zeros((128, 128), np.float32)
    for m in range(128):
        sel[32 * (m // 32) + 16, m] = 1.0
    return sel


def _split_multiwaits(nc):
    """Post-pass for walrus builds that accept only ONE sync-wait per
    instruction: split every instruction carrying N>1 waits into (N-1)
    single-wait NOPs on the same engine placed immediately before it."""
    uid = 0
    for f in nc.m.functions:
        for bb in f.blocks:
            il = bb.instructions
            if not any(
                i.sync_info is not None
                and i.sync_info.on_wait
                and len(i.sync_info.on_wait) > 1
                for i in il
            ):
                continue
            out = []
            for inst in il:
                si = inst.sync_info
                if si is not None and si.on_wait and len(si.on_wait) > 1:
                    waits = list(si.on_wait)
                    for w in waits[:-1]:
                        uid += 1
                        nop = mybir.InstNoOp(
                            name=f"WSPLIT-{uid}",
                            engine=inst.engine,
                            ins=[],
                            outs=[],
                            sync_info=mybir.SyncInfo(on_wait=[w], on_update=[]),
                        )
                        out.append(nop)
                    inst.sync_info = mybir.SyncInfo(
                        on_wait=[waits[-1]], on_update=list(si.on_update)
                    )
                out.append(inst)
            bb.instructions = out
    return nc


# ---------------------------------------------------------------- device build
def build_nc():
    nc = bass.Bass()

    def din(name, shape, dt=F32R):
        return nc.dram_tensor(name, list(shape), dt, kind="ExternalInput")

    xbT = din("xbT", (128, 2048))  # x_tgt[b]^T  (channels x keys)
    xqT = din("xqT", (128, 512))  # x_tgt query slice ^T
    eoT = din("eoT", (128, 512))  # enc_out query slice ^T
    wkp = [[din(f"wkp{a}{g}", (D, D)) for g in range(2)] for a in range(2)]
    wqp = [[din(f"wqp{a}{g}", (D, D)) for g in range(2)] for a in range(2)]
    wv_st = din("wv_st", (D, 512))  # [v_selfA | v_selfB | v_crossA | v_crossB]
    w1p = [din(f"w1p{g}", (D, D)) for g in range(2)]
    w2p = [din(f"w2p{g}", (D, D)) for g in range(2)]
    w3 = din("w3", (D, 512))
    w4r = din("w4r", (128, 512))  # col block j = w4[128j:128j+128, :]
    selt = din("selt", (128, 128))  # SEL[p, m] = (p == 32*(m//32)+16)
    ones_col = din("ones_col", (128, 1))
    ones_row = din("ones_row", (1, 128))
    b1t = din("b1t", (128, 1), F32)
    b2t = din("b2t", (128, 1), F32)
    b3t = din("b3t", (128, 4), F32)
    b4t = din("b4t", (128, 1), F32)
    y = nc.dram_tensor("y", [128, 512], F32, kind="ExternalOutput")

    with tile.TileContext(nc) as tc:
        with tc.tile_pool(name="persist", bufs=1) as pp:

            def sbuf(name, shape, dt=F32):
                return pp.tile(list(shape), dt, name=name, tag=name)

            def load(name, dram, shape, dt=F32R):
                t = sbuf(name, shape, dt)
                nc.sync.dma_start(out=t[:], in_=dram[:])
                return t

            # ---- constant / weight / activation loads
            xbT_t = load("xbT", xbT, (128, 2048))
            xqT_t = load("xqT", xqT, (128, 512))
            eoT_t = load("eoT", eoT, (128, 512))
            wkp_t = [[load(f"wkp{a}{g}", wkp[a][g], (D, D)) for g in range(2)]
                     for a in range(2)]
            wqp_t = [[load(f"wqp{a}{g}", wqp[a][g], (D, D)) for g in range(2)]
                     for a in range(2)]
            wv_t = load("wv", wv_st, (D, 512))
            w1p_t = [load(f"w1p{g}", w1p[g], (D, D)) for g in range(2)]
            w2p_t = [load(f"w2p{g}", w2p[g], (D, D)) for g in range(2)]
            w3_t = load("w3", w3, (D, 512))
            w4_t = load("w4", w4r, (128, 512))
            sel_t = load("sel", selt, (128, 128))
            onec_t = load("onec", ones_col, (128, 1))
            oner_t = load("oner", ones_row, (1, 128))
            b1_t = load("b1", b1t, (128, 1), F32)
            b2_t = load("b2", b2t, (128, 1), F32)
            b3_t = load("b3", b3t, (128, 4), F32)
            b4_t = load("b4", b4t, (128, 1), F32)

            # K^T packs: ktp[a][g] [128, 2048], partitions 32i+j = K^T row j
            # of head 4g+i.  Q^T packs: qp[a][g] [128, 512] same partition map.
            ktp = [[sbuf(f"ktp{a}{g}", (128, 2048), F32R) for g in range(2)]
                   for a in range(2)]
            qp = [[sbuf(f"qp{a}{g}", (128, 512), F32R) for g in range(2)]
                  for a in range(2)]
            v_all = sbuf("v_all", (128, 16, 512), BF16)

            # ---------------- setup phase: K/Q/V projections
            with tc.tile_pool(name="pset", bufs=3, space="PSUM") as pset:
                for a in range(2):
                    for g in range(2):
                        qsrc = xqT_t if a == 0 else eoT_t
                        qps = pset.tile([128, 512], F32, name=f"qps{a}{g}",
                                        tag="set")
                        nc.tensor.matmul(qps[:], lhsT=wqp_t[a][g][:],
                                         rhs=qsrc[:], start=True, stop=True)
                        nc.scalar.copy(out=qp[a][g][:], in_=qps[:])
                for t in range(KT):
                    vp = pset.tile([128, 512], F32, name=f"vp{t}", tag="set")
                    nc.tensor.matmul(
                        vp[:], lhsT=xbT_t[:, 128 * t: 128 * (t + 1)],
                        rhs=wv_t[:], start=True, stop=True)
                    nc.vector.tensor_copy(out=v_all[:, t, :], in_=vp[:])
                for a in range(2):
                    for g in range(2):
                        for ch in range(4):
                            kps = pset.tile([128, 512], F32,
                                            name=f"kps{a}{g}{ch}", tag="set")
                            nc.tensor.matmul(
                                kps[:], lhsT=wkp_t[a][g][:],
                                rhs=xbT_t[:, 512 * ch: 512 * (ch + 1)],
                                start=True, stop=True)
                            nc.scalar.copy(
                                out=ktp[a][g][:, 512 * ch: 512 * (ch + 1)],
                                in_=kps[:])
                # ones columns for the softmax-denominator rows
                nc.vector.memset(
                    v_all[:].rearrange("p t (c x) -> p t c x", x=32)[:, :, :, 16],
                    1.0,
                )

            # ---------------- attention loops
            def attention(ai, pa):
                """8 heads as 2 quads x 2 pairs; per (pair, k-tile): 8
                tile-positioned score matmuls (2 heads x 4 col-tiles of 32
                keys, K=16 contraction) into one [128,1024] PSUM tile, one
                exp (engine per EXP_SCHED), then 2 col-tiled PV matmuls
                accumulating 4 heads into one PSUM bank per quad."""
                packed = [pp.tile([128, 512], F32R, name=f"pk{ai}{g}",
                                  tag=f"pk{g}") for g in range(2)]
                with tc.tile_pool(name=f"ebp{ai}", bufs=3) as ebp:
                    for g in range(2):
                        for half in range(2):
                            pv = [pa.tile([32, 512], F32,
                                          name=f"pv{ai}{g}{half}{i}",
                                          tag=f"pv{i}") for i in range(2)]
                            for t in range(KT):
                                sc = pa.tile([128, 1024], F32, bufs=2,
                                             name=f"sc{ai}{g}{half}{t}",
                                             tag="sc")
                                for i in range(2):
                                    r = 2 * half + i
                                    nc.tensor.matmul(
                                        sc[:, 512 * i: 512 * (i + 1)],
                                        lhsT=ktp[ai][g][
                                            32 * r: 32 * r + HD,
                                            128 * t: 128 * (t + 1)],
                                        rhs=qp[ai][g][32 * r: 32 * r + HD, :],
                                        start=True, stop=True,
                                        tile_position=(32 * r, 0),
                                        skip_group_check=True,
                                    )
                                eb = ebp.tile([128, 1024], BF16, name="eb",
                                              tag="eb")
                                eng = EXP_SCHED[t]
                                if eng == "S":
                                    nc.scalar.activation(eb[:], sc[:], AF.Exp,
                                                         scale=0.25)
                                else:
                                    e = nc.vector if eng == "V" else nc.gpsimd
                                    e.tensor_scalar(
                                        out=eb[:].bitcast(I16), in0=sc[:],
                                        scalar1=SCHRA_A, scalar2=SCHRA_B,
                                        op0=OP.mult, op1=OP.add)
                                for i in range(2):
                                    h4 = 2 * half + i  # head within quad
                                    v0 = 256 * ai + 128 * g + 32 * h4
                                    nc.tensor.matmul(
                                        pv[i][:],
                                        lhsT=v_all[:, t, v0: v0 + 32],
                                        rhs=eb[:, 512 * i: 512 * (i + 1)],
                                        start=(t == 0), stop=(t == KT - 1),
                                        skip_group_check=True,
                                    )
                            for i in range(2):
                                h4 = 2 * half + i
                                nc.vector.tensor_copy(
                                    out=packed[g][32 * h4: 32 * h4 + 32, :],
                                    in_=pv[i][:])
                return packed

            def normalize_and_project(ai, pv, wp_t, res_prev, b_t, pa):
                """softmax-normalize the packed PV accumulators and apply the
                merge projection; returns r = proj + res_prev + b."""
                sa_n = []
                for grp in range(2):
                    sbc = pa.tile([128, 512], F32, name=f"sbc{ai}{grp}",
                                  tag=f"ps{grp}")
                    nc.tensor.matmul(sbc[:], lhsT=sel_t[:], rhs=pv[grp][:],
                                     start=True, stop=True)
                    rb = pp.tile([128, 512], F32, name=f"rb{ai}{grp}",
                                 tag=f"rb{grp}")
                    nc.vector.reciprocal(out=rb[:], in_=sbc[:])
                    sn = pp.tile([128, 512], F32R, name=f"sn{ai}{grp}",
                                 tag=f"sn{grp}")
                    nc.vector.tensor_mul(sn[:], pv[grp][:], rb[:])
                    sa_n.append(sn)
                rp = pa.tile([128, 512], F32, name=f"rp{ai}", tag="ps0")
                for grp in range(2):
                    nc.tensor.matmul(rp[:], lhsT=wp_t[grp][:],
                                     rhs=sa_n[grp][:],
                                     start=(grp == 0), stop=(grp == 1))
                rT = sbuf(f"r{ai}T", (128, 512), F32R)
                nc.vector.tensor_add(rT[:], rp[:], res_prev[:])
                nc.vector.tensor_scalar_add(rT[:], rT[:], b_t[:, 0:1])
                return rT

            with tc.tile_pool(name="pattn", bufs=1, space="PSUM") as pa:
                pv_s = attention(0, pa)
                r1T = normalize_and_project(0, pv_s, w1p_t, xqT_t, b1_t, pa)
                pv_c = attention(1, pa)
                r2T = normalize_and_project(1, pv_c, w2p_t, r1T, b2_t, pa)

            # ---------------- layernorm (x - m) / var, var unbiased
            with tc.tile_pool(name="ptail", bufs=1, space="PSUM") as pt:
                sq = sbuf("sq", (128, 512), F32R)
                nc.vector.tensor_mul(sq[:], r2T[:], r2T[:])
                mp = pt.tile([1, 512], F32, name="mp", tag="st0")
                nc.tensor.matmul(mp[:], lhsT=onec_t[:], rhs=r2T[:],
                                 start=True, stop=True)
                sp = pt.tile([1, 512], F32, name="sp", tag="st1")
                nc.tensor.matmul(sp[:], lhsT=onec_t[:], rhs=sq[:],
                                 start=True, stop=True)
                msb = sbuf("msb", (1, 512))
                nc.vector.tensor_copy(out=msb[:], in_=mp[:])
                t0 = sbuf("t0", (1, 512))
                # t0 = (mp * -1/128) * mp + sp = sum((x-m)^2)
                nc.vector.tensor_scalar(out=t0[:], in0=msb[:],
                                        scalar1=-1.0 / 128, scalar2=None,
                                        op0=OP.mult)
                nc.vector.tensor_mul(t0[:], t0[:], msb[:])
                nc.vector.tensor_add(t0[:], t0[:], sp[:])
                asb_f = sbuf("asb_f", (1, 512))
                nc.vector.reciprocal(out=asb_f[:], in_=t0[:])
                asb = sbuf("asb", (1, 512), F32R)
                nc.vector.tensor_scalar_mul(asb[:], asb_f[:], 127.0)  # 1/var
                bsb = sbuf("bsb", (1, 512), F32R)
                nc.vector.tensor_mul(bsb[:], msb[:], asb[:])
                nc.vector.tensor_scalar_mul(bsb[:], bsb[:], -1.0 / 128)
                abc = pt.tile([128, 512], F32, name="abc", tag="bc0")
                nc.tensor.matmul(abc[:], lhsT=oner_t[:], rhs=asb[:],
                                 start=True, stop=True)
                bbc = pt.tile([128, 512], F32, name="bbc", tag="bc1")
                nc.tensor.matmul(bbc[:], lhsT=oner_t[:], rhs=bsb[:],
                                 start=True, stop=True)
                lnT = sbuf("lnT", (128, 512), F32R)
                nc.vector.tensor_mul(lnT[:], r2T[:], abc[:])
                nc.vector.tensor_add(lnT[:], lnT[:], bbc[:])

                # ---------------- FFN
                h_sb = []
                for j in range(4):
                    hp = pt.tile([128, 512], F32, name=f"hp{j}", tag=f"hp{j % 2}")
                    nc.tensor.matmul(hp[:],
                                     lhsT=w3_t[:, 128 * j: 128 * (j + 1)],
                                     rhs=lnT[:], start=True, stop=True)
                    hs = sbuf(f"hs{j}", (128, 512), F32R)
                    nc.vector.tensor_scalar(
                        out=hs[:], in0=hp[:], scalar1=b3_t[:, j: j + 1],
                        scalar2=0.0, op0=OP.add, op1=OP.max,
                    )
                    h_sb.append(hs)
                op_ = pt.tile([128, 512], F32, name="op", tag="bc0")
                for j in range(4):
                    nc.tensor.matmul(op_[:],
                                     lhsT=w4_t[:, 128 * j: 128 * (j + 1)],
                                     rhs=h_sb[j][:],
                                     start=(j == 0), stop=(j == 3),
                                     skip_group_check=True)
                oT = sbuf("oT", (128, 512))
                nc.vector.tensor_add(oT[:], op_[:], r2T[:])
                nc.vector.tensor_scalar_add(oT[:], oT[:], b4_t[:, 0:1])
                nc.sync.dma_start(out=y[:], in_=oT[:])

    return nc


_CACHED = {}


def _get_nc():
    if "nc" not in _CACHED:
        _CACHED["nc"] = _split_multiwaits(build_nc())
    return _CACHED["nc"]


def _host_inputs(x_tgt, enc_out, self_wq, self_wk, self_wv, cross_wq, cross_wk,
                 cross_wv, w1, b1, w2, b2, w3, b3, w4, b4):
    shared = {
        "wkp00": _wk_quad(self_wk, 0), "wkp01": _wk_quad(self_wk, 1),
        "wkp10": _wk_quad(cross_wk, 0), "wkp11": _wk_quad(cross_wk, 1),
        "wqp00": _wk_quad(self_wq, 0), "wqp01": _wk_quad(self_wq, 1),
        "wqp10": _wk_quad(cross_wq, 0), "wqp11": _wk_quad(cross_wq, 1),
        "wv_st": np.concatenate(
            [_pack32_cols(self_wv, 0), _pack32_cols(self_wv, 1),
             _pack32_cols(cross_wv, 0), _pack32_cols(cross_wv, 1)], axis=1
        ),
        "w1p0": _pack_w12(w1, 0), "w1p1": _pack_w12(w1, 1),
        "w2p0": _pack_w12(w2, 0), "w2p1": _pack_w12(w2, 1),
        "w3": w3,
        "w4r": np.ascontiguousarray(
            w4.reshape(4, 128, 128).transpose(1, 0, 2).reshape(128, 512)
        ),
        "selt": _sel_matrix(),
        "ones_col": np.ones((128, 1), np.float32),
        "ones_row": np.ones((1, 128), np.float32),
        "b1t": b1.reshape(128, 1),
        "b2t": b2.reshape(128, 1),
        "b3t": np.ascontiguousarray(b3.reshape(4, 128).T),
        "b4t": b4.reshape(128, 1),
    }
    shared = {k: np.ascontiguousarray(v, dtype=np.float32)
              for k, v in shared.items()}
    in_maps = []
    for c in range(NCORES):
        b, qb = divmod(c, 4)
        q0 = qb * QC
        im = dict(shared)
        im["xbT"] = np.ascontiguousarray(x_tgt[b].T)
        im["xqT"] = np.ascontiguousarray(x_tgt[b, q0: q0 + QC].T)
        im["eoT"] = np.ascontiguousarray(enc_out[b, q0: q0 + QC].T)
        in_maps.append(im)
    return in_maps


def _unshuf(y):
    """device y [128, 512] is out^T for this core's query slice."""
    return np.ascontiguousarray(y.T)


def run_on_device(in_maps, **kw):
    nc = _get_nc()
    return run_bass_kernel_spmd(nc, in_maps, list(range(NCORES)), **kw)


def kernel(x_tgt, enc_out, self_wq, self_wk, self_wv, cross_wq, cross_wk,
           cross_wv, w1, b1, w2, b2, w3, b3, w4, b4, mask_src=None,
           mask_tgt=None, **_unused):
    args = [x_tgt, enc_out, self_wq, self_wk, self_wv, cross_wq, cross_wk,
            cross_wv, w1, b1, w2, b2, w3, b3, w4, b4]
    args = [np.asarray(a, dtype=np.float32) for a in args]
    in_maps = _host_inputs(*args)
    res = run_on_device(in_maps)
    out = np.empty((B, S, D), np.float32)
    for c in range(NCORES):
        b, qb = divmod(c, 4)
        out[b, qb * QC: (qb + 1) * QC] = _unshuf(res.results[c]["y"])
    return out


# revision 15
# speedup vs baseline: 1.1617x; 1.1617x over previous
"""Trainium2 Bass kernel for a (quirky) transformer decoder layer.

Problem shapes: B=2, S=2048, D=128, H=8 heads, head_dim=16.
  sa  = attn(q=x_tgt, kv=x_tgt);  r1 = sa @ w1 + b1 + x_tgt
  ca  = attn(q=enc_out, kv=x_tgt); r2 = ca @ w2 + b2 + r1
  ln  = (r2 - mean) / var   (var unbiased, divide by var not std)
  out = relu(ln @ w3 + b3) @ w4 + b4 + r2
(mask_src / mask_tgt are unused by the reference.)

Sharding: 8 cores, query-row sharding (zero communication). Core c handles
batch c//4, query rows [(c%4)*512 : (c%4+1)*512]. K/V are computed per-core
from the full 2048-row x_tgt of its batch.

v2 design notes (vs the 272us baseline):
- All activations arrive pre-transposed from the host ([d, token]); the
  on-device transpose phase is gone, and the output leaves transposed (host
  untransposes).
- Scores are computed per head with K=16 contraction (lhsT = K_h^T tile,
  rhs = Q_h^T) instead of the K=128 fused-G trick.  K^T/Q^T for 4 heads are
  packed on 32-partition boundaries so score matmuls for 2 heads x 4
  32-key col-tiles co-execute on the PE array via tile_position (the
  128x128 array is 16 independent 32x32 subarrays).
- PV uses col-tiling: each head's [128k,32v] PV matmul targets a 32-col
  slice of one PSUM bank, so 2 heads run concurrently and 4 heads
  accumulate into one bank, which directly produces the packed layout the
  merge projection wants.  The packed V carries a ones-column per head for
  the softmax denominator (as in the baseline).
- exp is split between ScalarE (true Exp activation) and VectorE/GpSimdE
  (Schraudolph bit-trick: round(s*a + b) as int32 IS fp32 exp(s/4) to
  ~2% relative, which softmax normalization mostly cancels).
- reciprocal -> reciprocal_approx_fast (5x).
"""

import ml_dtypes
import numpy as np

import concourse.bass as bass
import concourse.tile as tile
from concourse import mybir
from concourse.bass_utils import run_bass_kernel_spmd

B, S, D, H, HD = 2, 2048, 128, 8, 16
QC = 512  # query rows per core
NCORES = 8
KT = 16  # number of 128-row key tiles
F32 = mybir.dt.float32
F32R = mybir.dt.float32r
I16 = mybir.dt.int16
BF16 = mybir.dt.bfloat16
AF = mybir.ActivationFunctionType
OP = mybir.AluOpType

# Schraudolph exp(s/4) constants: int16 bits of bf16 = s * (0.25 * 2^7/ln2) + B
SCHRA_A = 0.25 * (2.0**7) / np.log(2.0)
SCHRA_B = 127.0 * 2.0**7 - 0.0579 * 2.0**7

PV_SPLIT = False  # split PV over key halves on distinct quadrant rows
WARMUP = True  # HAM warm-up burst

# exp engine schedule per k-tile (16 entries): S=ScalarE, V=VectorE, P=GpSimd
EXP_SCHED = "SSVSSSVSSSVSSSVS"


# ---------------------------------------------------------------- host packing
def _wk_quad(w, g):
    """[128, 128] lhsT for the K^T projection of head quad g: out partition
    32i+j (i<4, j<16) = K^T row j of head 4g+i, i.e. col 32i+j = w[:, j*H + 4g+i]."""
    out = np.zeros((D, 128), np.float32)
    for i in range(4):
        for j in range(HD):
            out[:, 32 * i + j] = w[:, j * H + 4 * g + i]
    return out


def _pack32_cols(w, grp):
    """[D, 128]: col 32g+j (j<16) = w[:, j*H + (4*grp+g)], else 0 (V packing;
    col 32g+16 stays 0 and is later set to 1 for the softmax denominator)."""
    out = np.zeros((D, 128), np.float32)
    for g in range(4):
        h = 4 * grp + g
        for j in range(HD):
            out[:, 32 * g + j] = w[:, j * H + h]
    return out


def _pack_w12(w, grp):
    """lhsT for the merge projection: row 32c+j = w[j*H + (4*grp+c), :]."""
    out = np.zeros((D, D), np.float32)
    for c in range(4):
        h = 4 * grp + c
        for j in range(HD):
            out[32 * c + j, :] = w[j * H + h, :]
    return out


def _sel_matrix():
    sel = np.zeros((128, 128), np.float32)
    for m in range(128):
        sel[32 * (m // 32) + 16, m] = 1.0
    return sel


def _split_multiwaits(nc):
    """Post-pass for walrus builds that accept only ONE sync-wait per
    instruction: split every instruction carrying N>1 waits into (N-1)
    single-wait NOPs on the same engine placed immediately before it."""
    uid = 0
    for f in nc.m.functions:
        for bb in f.blocks:
            il = bb.instructions
            if not any(
                i.sync_info is not None
                and i.sync_info.on_wait
                and len(i.sync_info.on_wait) > 1
                for i in il
            ):
                continue
            out = []
            for inst in il:
                si = inst.sync_info
                if si is not None and si.on_wait and len(si.on_wait) > 1:
                    waits = list(si.on_wait)
                    for w in waits[:-1]:
                        uid += 1
                        nop = mybir.InstNoOp(
                            name=f"WSPLIT-{uid}",
                            engine=inst.engine,
                            ins=[],
                            outs=[],
                            sync_info=mybir.SyncInfo(on_wait=[w], on_update=[]),
                        )
                        out.append(nop)
                    inst.sync_info = mybir.SyncInfo(
                        on_wait=[waits[-1]], on_update=list(si.on_update)
                    )
                out.append(inst)
            bb.instructions = out
    return nc


# ---------------------------------------------------------------- device build
def build_nc():
    nc = bass.Bass()

    def din(name, shape, dt=F32R):
        return nc.dram_tensor(name, list(shape), dt, kind="ExternalInput")

    xbT = din("xbT", (128, 2048), BF16)  # x_tgt[b]^T  (channels x keys)
    xqT = din("xqT", (128, 512))  # x_tgt query slice ^T
    eoT = din("eoT", (128, 512))  # enc_out query slice ^T
    wkp = [[din(f"wkp{a}{g}", (D, D), BF16) for g in range(2)] for a in range(2)]
    wqp = [[din(f"wqp{a}{g}", (D, D)) for g in range(2)] for a in range(2)]
    wv_st = din("wv_st", (D, 512), BF16)  # [v_selfA | v_selfB | v_crossA | v_crossB]
    w1p = [din(f"w1p{g}", (D, D)) for g in range(2)]
    w2p = [din(f"w2p{g}", (D, D)) for g in range(2)]
    w3 = din("w3", (D, 512))
    w4r = din("w4r", (128, 512))  # col block j = w4[128j:128j+128, :]
    selt = din("selt", (128, 128))  # SEL[p, m] = (p == 32*(m//32)+16)
    ones_col = din("ones_col", (128, 1))
    ones_row = din("ones_row", (1, 128))
    b1t = din("b1t", (128, 1), F32)
    b2t = din("b2t", (128, 1), F32)
    b3t = din("b3t", (128, 4), F32)
    b4t = din("b4t", (128, 1), F32)
    y = nc.dram_tensor("y", [128, 512], F32, kind="ExternalOutput")

    with tile.TileContext(nc) as tc:
        with tc.tile_pool(name="persist", bufs=1) as pp:

            def sbuf(name, shape, dt=F32):
                return pp.tile(list(shape), dt, name=name, tag=name)

            def load(name, dram, shape, dt=F32R):
                t = sbuf(name, shape, dt)
                nc.sync.dma_start(out=t[:], in_=dram[:])
                return t

            # ---- constant / weight / activation loads (DMA queue order
            # matters: small projection weights first so setup matmuls can
            # start while the big xbT halves stream in)
            wv_t = load("wv", wv_st, (D, 512), BF16)
            wqp_t = [[load(f"wqp{a}{g}", wqp[a][g], (D, D)) for g in range(2)]
                     for a in range(2)]
            xqT_t = load("xqT", xqT, (128, 512))
            eoT_t = load("eoT", eoT, (128, 512))
            wkp_t = [[load(f"wkp{a}{g}", wkp[a][g], (D, D), BF16)
                      for g in range(2)] for a in range(2)]
            xbT_t = sbuf("xbT", (128, 2048), BF16)
            nc.sync.dma_start(out=xbT_t[:, 0:1024], in_=xbT[:, 0:1024])
            nc.sync.dma_start(out=xbT_t[:, 1024:2048], in_=xbT[:, 1024:2048])
            w1p_t = [load(f"w1p{g}", w1p[g], (D, D)) for g in range(2)]
            w2p_t = [load(f"w2p{g}", w2p[g], (D, D)) for g in range(2)]
            w3_t = load("w3", w3, (D, 512))
            w4_t = load("w4", w4r, (128, 512))
            sel_t = load("sel", selt, (128, 128))
            onec_t = load("onec", ones_col, (128, 1))
            oner_t = load("oner", ones_row, (1, 128))
            b1_t = load("b1", b1t, (128, 1), F32)
            b2_t = load("b2", b2t, (128, 1), F32)
            b3_t = load("b3", b3t, (128, 4), F32)
            b4_t = load("b4", b4t, (128, 1), F32)

            # HAM warm-up: a burst of dependency-free back-to-back matmuls
            # so the PE clock-gate opens (K=8/8) before the real work; the
            # PE otherwise idles on DMAs early and runs the whole kernel at
            # half clock.
            if WARMUP:
                wsrc = sbuf("wsrc", (128, 128), BF16)
                nc.vector.memset(wsrc[:], 0.0)
                with tc.tile_pool(name="pwarm", bufs=2, space="PSUM") as pw:
                    for w in range(40):
                        wt = pw.tile([128, 128], F32, name=f"warm{w}", tag="wm")
                        nc.tensor.matmul(wt[:], lhsT=wsrc[:], rhs=wsrc[:],
                                         start=True, stop=True,
                                         skip_group_check=True)

            # K^T packs: ktp[a][g] [128, 2048], partitions 32i+j = K^T row j
            # of head 4g+i.  Q^T packs: qp[a][g] [128, 512] same partition map.
            ktp = [[sbuf(f"ktp{a}{g}", (128, 2048), BF16) for g in range(2)]
                   for a in range(2)]
            qp = [[sbuf(f"qp{a}{g}", (128, 512), BF16) for g in range(2)]
                  for a in range(2)]
            v_all = sbuf("v_all", (128, 16, 512), BF16)

            # ---------------- setup phase: K/Q/V projections
            with tc.tile_pool(name="pset", bufs=3, space="PSUM") as pset:
                for a in range(2):
                    for g in range(2):
                        qsrc = xqT_t if a == 0 else eoT_t
                        qps = pset.tile([128, 512], F32, name=f"qps{a}{g}",
                                        tag="set")
                        nc.tensor.matmul(qps[:], lhsT=wqp_t[a][g][:],
                                         rhs=qsrc[:], start=True, stop=True)
                        nc.scalar.copy(out=qp[a][g][:], in_=qps[:])
                for a in range(2):
                    for g in range(2):
                        for ch in range(4):
                            kps = pset.tile([128, 512], F32,
                                            name=f"kps{a}{g}{ch}", tag="set")
                            nc.tensor.matmul(
                                kps[:], lhsT=wkp_t[a][g][:],
                                rhs=xbT_t[:, 512 * ch: 512 * (ch + 1)],
                                start=True, stop=True)
                            nc.scalar.copy(
                                out=ktp[a][g][:, 512 * ch: 512 * (ch + 1)],
                                in_=kps[:])
                for t in range(KT):
                    vp = pset.tile([128, 512], F32, name=f"vp{t}", tag="set")
                    nc.tensor.matmul(
                        vp[:], lhsT=xbT_t[:, 128 * t: 128 * (t + 1)],
                        rhs=wv_t[:], start=True, stop=True)
                    # softmax-denominator ones column per 32-col head block
                    nc.vector.memset(
                        vp[:].rearrange("p (c x) -> p c x", x=32)[:, :, 16],
                        1.0)
                    nc.vector.tensor_copy(out=v_all[:, t, :], in_=vp[:])

            # ---------------- attention loops
            def attention(ai, pa):
                """8 heads as 2 quads x 2 pairs; per (pair, k-tile): 8
                tile-positioned score matmuls (2 heads x 4 col-tiles of 32
                keys, K=16 contraction) into one [128,1024] PSUM tile, one
                exp (engine per EXP_SCHED), then 2 col-tiled PV matmuls
                accumulating 4 heads into one PSUM bank per quad."""
                packed = [pp.tile([128, 512], F32R, name=f"pk{ai}{g}",
                                  tag=f"pk{ai}{g}") for g in range(2)]
                with tc.tile_pool(name=f"ebp{ai}", bufs=3) as ebp:
                    for g in range(2):
                        for half in range(2):
                            # per head: PV split into lo/hi key halves on
                            # different quadrant rows; waves [A-lo|B-hi] and
                            # [A-hi|B-lo] co-execute on disjoint rows
                            pv = [pa.tile([32, 512], F32,
                                          name=f"pv{ai}{g}{half}{i}",
                                          tag=f"pv{i}") for i in range(2)]
                            for t in range(KT):
                                sc = pa.tile([128, 1024], F32, bufs=3,
                                             name=f"sc{ai}{g}{half}{t}",
                                             tag="sc")
                                for i in range(2):
                                    r = 2 * half + i
                                    nc.tensor.matmul(
                                        sc[:, 512 * i: 512 * (i + 1)],
                                        lhsT=ktp[ai][g][
                                            32 * r: 32 * r + HD,
                                            128 * t: 128 * (t + 1)],
                                        rhs=qp[ai][g][32 * r: 32 * r + HD, :],
                                        start=True, stop=True,
                                        tile_position=(32 * r, 0),
                                        skip_group_check=True,
                                    )
                                eb = ebp.tile([128, 1024], BF16, name="eb",
                                              tag="eb")
                                eng = EXP_SCHED[t]
                                if eng == "S":
                                    nc.scalar.activation(eb[:], sc[:], AF.Exp,
                                                         scale=0.25)
                                else:
                                    e = nc.vector if eng == "V" else nc.gpsimd
                                    e.tensor_scalar(
                                        out=eb[:].bitcast(I16), in0=sc[:],
                                        scalar1=SCHRA_A, scalar2=SCHRA_B,
                                        op0=OP.mult, op1=OP.add)
                                v0 = [256 * ai + 128 * g + 32 * (2 * half + i)
                                      for i in range(2)]
                                if PV_SPLIT:
                                    for i, lo in ((0, 0), (1, 1), (0, 1), (1, 0)):
                                        nc.tensor.matmul(
                                            pv[i][:],
                                            lhsT=v_all[64 * lo: 64 * (lo + 1),
                                                       t, v0[i]: v0[i] + 32],
                                            rhs=eb[64 * lo: 64 * (lo + 1),
                                                   512 * i: 512 * (i + 1)],
                                            start=(t == 0 and i == lo),
                                            stop=(t == KT - 1 and i != lo),
                                            tile_position=(64 * lo, 0),
                                            skip_group_check=True,
                                        )
                                else:
                                    for i in range(2):
                                        nc.tensor.matmul(
                                            pv[i][:],
                                            lhsT=v_all[:, t, v0[i]: v0[i] + 32],
                                            rhs=eb[:, 512 * i: 512 * (i + 1)],
                                            start=(t == 0), stop=(t == KT - 1),
                                            skip_group_check=True,
                                        )
                            for i in range(2):
                                h4 = 2 * half + i
                                nc.vector.tensor_copy(
                                    out=packed[g][32 * h4: 32 * h4 + 32, :],
                                    in_=pv[i][:])
                return packed

            def normalize_and_project(ai, pv, wp_t, res_prev, b_t, pa):
                """softmax-normalize the packed PV accumulators and apply the
                merge projection; returns r = proj + res_prev + b."""
                sa_n = []
                for grp in range(2):
                    sbc = pa.tile([128, 512], F32, name=f"sbc{ai}{grp}",
                                  tag=f"ps{grp}")
                    nc.tensor.matmul(sbc[:], lhsT=sel_t[:], rhs=pv[grp][:],
                                     start=True, stop=True)
                    rb = pp.tile([128, 512], F32, name=f"rb{ai}{grp}",
                                 tag=f"rb{grp}")
                    nc.vector.reciprocal(out=rb[:], in_=sbc[:])
                    sn = pp.tile([128, 512], F32R, name=f"sn{ai}{grp}",
                                 tag=f"sn{grp}")
                    nc.vector.tensor_mul(sn[:], pv[grp][:], rb[:])
                    sa_n.append(sn)
                rp = pa.tile([128, 512], F32, name=f"rp{ai}", tag="ps0")
                for grp in range(2):
                    nc.tensor.matmul(rp[:], lhsT=wp_t[grp][:],
                                     rhs=sa_n[grp][:],
                                     start=(grp == 0), stop=(grp == 1))
                rT = sbuf(f"r{ai}T", (128, 512), F32R)
                nc.vector.tensor_add(rT[:], rp[:], res_prev[:])
                nc.vector.tensor_scalar_add(rT[:], rT[:], b_t[:, 0:1])
                return rT

            with tc.tile_pool(name="pattn", bufs=1, space="PSUM") as pa:
                pv_s = attention(0, pa)
                pv_c = attention(1, pa)
            with tc.tile_pool(name="pnorm", bufs=1, space="PSUM") as pa:
                r1T = normalize_and_project(0, pv_s, w1p_t, xqT_t, b1_t, pa)
                r2T = normalize_and_project(1, pv_c, w2p_t, r1T, b2_t, pa)

            # ---------------- layernorm (x - m) / var, var unbiased
            with tc.tile_pool(name="ptail", bufs=1, space="PSUM") as pt:
                sq = sbuf("sq", (128, 512), F32R)
                nc.vector.tensor_mul(sq[:], r2T[:], r2T[:])
                mp = pt.tile([1, 512], F32, name="mp", tag="st0")
                nc.tensor.matmul(mp[:], lhsT=onec_t[:], rhs=r2T[:],
                                 start=True, stop=True)
                sp = pt.tile([1, 512], F32, name="sp", tag="st1")
                nc.tensor.matmul(sp[:], lhsT=onec_t[:], rhs=sq[:],
                                 start=True, stop=True)
                msb = sbuf("msb", (1, 512))
                nc.vector.tensor_copy(out=msb[:], in_=mp[:])
                t0 = sbuf("t0", (1, 512))
                # t0 = (mp * -1/128) * mp + sp = sum((x-m)^2)
                nc.vector.tensor_scalar(out=t0[:], in0=msb[:],
                                        scalar1=-1.0 / 128, scalar2=None,
                                        op0=OP.mult)
                nc.vector.tensor_mul(t0[:], t0[:], msb[:])
                nc.vector.tensor_add(t0[:], t0[:], sp[:])
                asb_f = sbuf("asb_f", (1, 512))
                nc.vector.reciprocal(out=asb_f[:], in_=t0[:])
                asb = sbuf("asb", (1, 512), F32R)
                nc.vector.tensor_scalar_mul(asb[:], asb_f[:], 127.0)  # 1/var
                bsb = sbuf("bsb", (1, 512), F32R)
                nc.vector.tensor_mul(bsb[:], msb[:], asb[:])
                nc.vector.tensor_scalar_mul(bsb[:], bsb[:], -1.0 / 128)
                abc = pt.tile([128, 512], F32, name="abc", tag="bc0")
                nc.tensor.matmul(abc[:], lhsT=oner_t[:], rhs=asb[:],
                                 start=True, stop=True)
                bbc = pt.tile([128, 512], F32, name="bbc", tag="bc1")
                nc.tensor.matmul(bbc[:], lhsT=oner_t[:], rhs=bsb[:],
                                 start=True, stop=True)
                lnT = sbuf("lnT", (128, 512), F32R)
                nc.vector.tensor_mul(lnT[:], r2T[:], abc[:])
                nc.vector.tensor_add(lnT[:], lnT[:], bbc[:])

                # ---------------- FFN
                h_sb = []
                for j in range(4):
                    hp = pt.tile([128, 512], F32, name=f"hp{j}", tag=f"hp{j % 2}")
                    nc.tensor.matmul(hp[:],
                                     lhsT=w3_t[:, 128 * j: 128 * (j + 1)],
                                     rhs=lnT[:], start=True, stop=True)
                    hs = sbuf(f"hs{j}", (128, 512), F32R)
                    nc.vector.tensor_scalar(
                        out=hs[:], in0=hp[:], scalar1=b3_t[:, j: j + 1],
                        scalar2=0.0, op0=OP.add, op1=OP.max,
                    )
                    h_sb.append(hs)
                op_ = pt.tile([128, 512], F32, name="op", tag="bc0")
                for j in range(4):
                    nc.tensor.matmul(op_[:],
                                     lhsT=w4_t[:, 128 * j: 128 * (j + 1)],
                                     rhs=h_sb[j][:],
                                     start=(j == 0), stop=(j == 3),
                                     skip_group_check=True)
                oT = sbuf("oT", (128, 512))
                nc.vector.tensor_add(oT[:], op_[:], r2T[:])
                nc.vector.tensor_scalar_add(oT[:], oT[:], b4_t[:, 0:1])
                nc.sync.dma_start(out=y[:], in_=oT[:])

    return nc


_CACHED = {}


def _get_nc():
    if "nc" not in _CACHED:
        _CACHED["nc"] = _split_multiwaits(build_nc())
    return _CACHED["nc"]


def _host_inputs(x_tgt, enc_out, self_wq, self_wk, self_wv, cross_wq, cross_wk,
                 cross_wv, w1, b1, w2, b2, w3, b3, w4, b4):
    shared = {
        "wkp00": _wk_quad(self_wk, 0), "wkp01": _wk_quad(self_wk, 1),
        "wkp10": _wk_quad(cross_wk, 0), "wkp11": _wk_quad(cross_wk, 1),
        "wqp00": _wk_quad(self_wq, 0), "wqp01": _wk_quad(self_wq, 1),
        "wqp10": _wk_quad(cross_wq, 0), "wqp11": _wk_quad(cross_wq, 1),
        "wv_st": np.concatenate(
            [_pack32_cols(self_wv, 0), _pack32_cols(self_wv, 1),
             _pack32_cols(cross_wv, 0), _pack32_cols(cross_wv, 1)], axis=1
        ),
        "w1p0": _pack_w12(w1, 0), "w1p1": _pack_w12(w1, 1),
        "w2p0": _pack_w12(w2, 0), "w2p1": _pack_w12(w2, 1),
        "w3": w3,
        "w4r": np.ascontiguousarray(
            w4.reshape(4, 128, 128).transpose(1, 0, 2).reshape(128, 512)
        ),
        "selt": _sel_matrix(),
        "ones_col": np.ones((128, 1), np.float32),
        "ones_row": np.ones((1, 128), np.float32),
        "b1t": b1.reshape(128, 1),
        "b2t": b2.reshape(128, 1),
        "b3t": np.ascontiguousarray(b3.reshape(4, 128).T),
        "b4t": b4.reshape(128, 1),
    }
    bf16 = {"wkp00", "wkp01", "wkp10", "wkp11", "wv_st"}
    shared = {k: np.ascontiguousarray(
        v, dtype=(ml_dtypes.bfloat16 if k in bf16 else np.float32))
              for k, v in shared.items()}
    in_maps = []
    for c in range(NCORES):
        b, qb = divmod(c, 4)
        q0 = qb * QC
        im = dict(shared)
        im["xbT"] = np.ascontiguousarray(x_tgt[b].T, dtype=ml_dtypes.bfloat16)
        im["xqT"] = np.ascontiguousarray(x_tgt[b, q0: q0 + QC].T)
        im["eoT"] = np.ascontiguousarray(enc_out[b, q0: q0 + QC].T)
        in_maps.append(im)
    return in_maps


def _unshuf(y):
    """device y [128, 512] is out^T for this core's query slice."""
    return np.ascontiguousarray(y.T)


def run_on_device(in_maps, **kw):
    nc = _get_nc()
    return run_bass_kernel_spmd(nc, in_maps, list(range(NCORES)), **kw)


def kernel(x_tgt, enc_out, self_wq, self_wk, self_wv, cross_wq, cross_wk,
           cross_wv, w1, b1, w2, b2, w3, b3, w4, b4, mask_src=None,
           mask_tgt=None, **_unused):
    args = [x_tgt, enc_out, self_wq, self_wk, self_wv, cross_wq, cross_wk,
            cross_wv, w1, b1, w2, b2, w3, b3, w4, b4]
    args = [np.asarray(a, dtype=np.float32) for a in args]
    in_maps = _host_inputs(*args)
    res = run_on_device(in_maps)
    out = np.empty((B, S, D), np.float32)
    for c in range(NCORES):
        b, qb = divmod(c, 4)
        out[b, qb * QC: (qb + 1) * QC] = _unshuf(res.results[c]["y"])
    return out


# revision 18
# speedup vs baseline: 1.2383x; 1.0659x over previous
"""Trainium2 Bass kernel for a (quirky) transformer decoder layer.

Problem shapes: B=2, S=2048, D=128, H=8 heads, head_dim=16.
  sa  = attn(q=x_tgt, kv=x_tgt);  r1 = sa @ w1 + b1 + x_tgt
  ca  = attn(q=enc_out, kv=x_tgt); r2 = ca @ w2 + b2 + r1
  ln  = (r2 - mean) / var   (var unbiased, divide by var not std)
  out = relu(ln @ w3 + b3) @ w4 + b4 + r2
(mask_src / mask_tgt are unused by the reference.)

Sharding: 8 cores, query-row sharding (zero communication). Core c handles
batch c//4, query rows [(c%4)*512 : (c%4+1)*512]. K/V are computed per-core
from the full 2048-row x_tgt of its batch.

v2 design notes (vs the 272us baseline):
- All activations arrive pre-transposed from the host ([d, token]); the
  on-device transpose phase is gone, and the output leaves transposed (host
  untransposes).
- Scores are computed per head with K=16 contraction (lhsT = K_h^T tile,
  rhs = Q_h^T) instead of the K=128 fused-G trick.  K^T/Q^T for 4 heads are
  packed on 32-partition boundaries so score matmuls for 2 heads x 4
  32-key col-tiles co-execute on the PE array via tile_position (the
  128x128 array is 16 independent 32x32 subarrays).
- PV uses col-tiling: each head's [128k,32v] PV matmul targets a 32-col
  slice of one PSUM bank, so 2 heads run concurrently and 4 heads
  accumulate into one bank, which directly produces the packed layout the
  merge projection wants.  The packed V carries a ones-column per head for
  the softmax denominator (as in the baseline).
- exp is split between ScalarE (true Exp activation) and VectorE/GpSimdE
  (Schraudolph bit-trick: round(s*a + b) as int32 IS fp32 exp(s/4) to
  ~2% relative, which softmax normalization mostly cancels).
- reciprocal -> reciprocal_approx_fast (5x).
"""

import ml_dtypes
import numpy as np

import concourse.bass as bass
import concourse.tile as tile
from concourse import mybir
from concourse.bass_utils import run_bass_kernel_spmd

B, S, D, H, HD = 2, 2048, 128, 8, 16
QC = 512  # query rows per core
NCORES = 8
KT = 16  # number of 128-row key tiles
F32 = mybir.dt.float32
F32R = mybir.dt.float32r
I16 = mybir.dt.int16
BF16 = mybir.dt.bfloat16
AF = mybir.ActivationFunctionType
OP = mybir.AluOpType

# Schraudolph exp(s/4) constants: int16 bits of bf16 = s * (0.25 * 2^7/ln2) + B
SCHRA_A = 0.25 * (2.0**7) / np.log(2.0)
SCHRA_B = 127.0 * 2.0**7 - 0.0579 * 2.0**7

PV_SPLIT = False  # split PV over key halves on distinct quadrant rows
WARMUP = True  # HAM warm-up burst

# exp engine schedule per k-tile (16 entries): S=ScalarE, V=VectorE, P=GpSimd
EXP_SCHED = "SSVSSSVSSSVSSSVS"


# ---------------------------------------------------------------- host packing
def _wk_quad(w, g):
    """[128, 128] lhsT for the K^T projection of head quad g: out partition
    32i+j (i<4, j<16) = K^T row j of head 4g+i, i.e. col 32i+j = w[:, j*H + 4g+i]."""
    out = np.zeros((D, 128), np.float32)
    for i in range(4):
        for j in range(HD):
            out[:, 32 * i + j] = w[:, j * H + 4 * g + i]
    return out


def _pack32_cols(w, grp):
    """[D, 128]: col 32g+j (j<16) = w[:, j*H + (4*grp+g)], else 0 (V packing;
    col 32g+16 stays 0 and is later set to 1 for the softmax denominator)."""
    out = np.zeros((D, 128), np.float32)
    for g in range(4):
        h = 4 * grp + g
        for j in range(HD):
            out[:, 32 * g + j] = w[:, j * H + h]
    return out


def _pack_w12(w, grp):
    """lhsT for the merge projection: row 32c+j = w[j*H + (4*grp+c), :]."""
    out = np.zeros((D, D), np.float32)
    for c in range(4):
        h = 4 * grp + c
        for j in range(HD):
            out[32 * c + j, :] = w[j * H + h, :]
    return out


def _sel_matrix():
    sel = np.zeros((128, 128), np.float32)
    for m in range(128):
        sel[32 * (m // 32) + 16, m] = 1.0
    return sel


def _split_multiwaits(nc):
    """Post-pass for walrus builds that accept only ONE sync-wait per
    instruction: split every instruction carrying N>1 waits into (N-1)
    single-wait NOPs on the same engine placed immediately before it."""
    uid = 0
    for f in nc.m.functions:
        for bb in f.blocks:
            il = bb.instructions
            if not any(
                i.sync_info is not None
                and i.sync_info.on_wait
                and len(i.sync_info.on_wait) > 1
                for i in il
            ):
                continue
            out = []
            for inst in il:
                si = inst.sync_info
                if si is not None and si.on_wait and len(si.on_wait) > 1:
                    waits = list(si.on_wait)
                    for w in waits[:-1]:
                        uid += 1
                        nop = mybir.InstNoOp(
                            name=f"WSPLIT-{uid}",
                            engine=inst.engine,
                            ins=[],
                            outs=[],
                            sync_info=mybir.SyncInfo(on_wait=[w], on_update=[]),
                        )
                        out.append(nop)
                    inst.sync_info = mybir.SyncInfo(
                        on_wait=[waits[-1]], on_update=list(si.on_update)
                    )
                out.append(inst)
            bb.instructions = out
    return nc


# ---------------------------------------------------------------- device build
def build_nc():
    nc = bass.Bass()

    def din(name, shape, dt=F32R):
        return nc.dram_tensor(name, list(shape), dt, kind="ExternalInput")

    xbT = din("xbT", (128, 2048), BF16)  # x_tgt[b]^T  (channels x keys)
    xqT = din("xqT", (128, 512))  # x_tgt query slice ^T
    eoT = din("eoT", (128, 512))  # enc_out query slice ^T
    wkp = [[din(f"wkp{a}{g}", (D, D), BF16) for g in range(2)] for a in range(2)]
    wqp = [[din(f"wqp{a}{g}", (D, D)) for g in range(2)] for a in range(2)]
    wv_st = din("wv_st", (D, 512), BF16)  # [v_selfA | v_selfB | v_crossA | v_crossB]
    w1p = [din(f"w1p{g}", (D, D)) for g in range(2)]
    w2p = [din(f"w2p{g}", (D, D)) for g in range(2)]
    w3 = din("w3", (D, 512))
    w4r = din("w4r", (128, 512))  # col block j = w4[128j:128j+128, :]
    selt = din("selt", (128, 128))  # SEL[p, m] = (p == 32*(m//32)+16)
    ones_col = din("ones_col", (128, 1))
    ones_row = din("ones_row", (1, 128))
    b1t = din("b1t", (128, 1), F32)
    b2t = din("b2t", (128, 1), F32)
    b3t = din("b3t", (128, 4), F32)
    b4t = din("b4t", (128, 1), F32)
    y = nc.dram_tensor("y", [128, 512], F32, kind="ExternalOutput")

    with tile.TileContext(nc) as tc:
        with tc.tile_pool(name="persist", bufs=1) as pp:

            def sbuf(name, shape, dt=F32):
                return pp.tile(list(shape), dt, name=name, tag=name)

            def load(name, dram, shape, dt=F32R):
                t = sbuf(name, shape, dt)
                nc.sync.dma_start(out=t[:], in_=dram[:])
                return t

            # ---- constant / weight / activation loads (DMA queue order
            # matters: small projection weights first so setup matmuls can
            # start while the big xbT halves stream in)
            wv_t = load("wv", wv_st, (D, 512), BF16)
            wqp_t = [[load(f"wqp{a}{g}", wqp[a][g], (D, D)) for g in range(2)]
                     for a in range(2)]
            xqT_t = load("xqT", xqT, (128, 512))
            eoT_t = load("eoT", eoT, (128, 512))
            wkp_t = [[load(f"wkp{a}{g}", wkp[a][g], (D, D), BF16)
                      for g in range(2)] for a in range(2)]
            xbT_t = sbuf("xbT", (128, 2048), BF16)
            nc.sync.dma_start(out=xbT_t[:, 0:1024], in_=xbT[:, 0:1024])
            nc.sync.dma_start(out=xbT_t[:, 1024:2048], in_=xbT[:, 1024:2048])
            w1p_t = [load(f"w1p{g}", w1p[g], (D, D)) for g in range(2)]
            w2p_t = [load(f"w2p{g}", w2p[g], (D, D)) for g in range(2)]
            w3_t = load("w3", w3, (D, 512))
            w4_t = load("w4", w4r, (128, 512))
            sel_t = load("sel", selt, (128, 128))
            onec_t = load("onec", ones_col, (128, 1))
            oner_t = load("oner", ones_row, (1, 128))
            b1_t = load("b1", b1t, (128, 1), F32)
            b2_t = load("b2", b2t, (128, 1), F32)
            b3_t = load("b3", b3t, (128, 4), F32)
            b4_t = load("b4", b4t, (128, 1), F32)

            # HAM warm-up: a burst of dependency-free back-to-back matmuls
            # so the PE clock-gate opens (K=8/8) before the real work; the
            # PE otherwise idles on DMAs early and runs the whole kernel at
            # half clock.
            if WARMUP:
                wsrc = sbuf("wsrc", (128, 128), BF16)
                nc.vector.memset(wsrc[:], 0.0)
                with tc.tile_pool(name="pwarm", bufs=2, space="PSUM") as pw:
                    for w in range(28):
                        wt = pw.tile([128, 128], F32, name=f"warm{w}", tag="wm")
                        nc.tensor.matmul(wt[:], lhsT=wsrc[:], rhs=wsrc[:],
                                         start=True, stop=True,
                                         skip_group_check=True)

            # K^T packs: ktp[a][g] [128, 2048], partitions 32i+j = K^T row j
            # of head 4g+i.  Q^T packs: qp[a][g] [128, 512] same partition map.
            ktp = [[sbuf(f"ktp{a}{g}", (128, 2048), BF16) for g in range(2)]
                   for a in range(2)]
            qp = [[sbuf(f"qp{a}{g}", (128, 512), BF16) for g in range(2)]
                  for a in range(2)]
            v_all = sbuf("v_all", (128, 16, 512), BF16)

            # ---------------- setup phase: K/Q/V projections
            with tc.tile_pool(name="pset", bufs=3, space="PSUM") as pset:
                for a in range(2):
                    for g in range(2):
                        qsrc = xqT_t if a == 0 else eoT_t
                        qps = pset.tile([128, 512], F32, name=f"qps{a}{g}",
                                        tag="set")
                        nc.tensor.matmul(qps[:], lhsT=wqp_t[a][g][:],
                                         rhs=qsrc[:], start=True, stop=True)
                        nc.scalar.copy(out=qp[a][g][:], in_=qps[:])
                for a in range(2):
                    for g in range(2):
                        for ch in range(4):
                            kps = pset.tile([128, 512], F32,
                                            name=f"kps{a}{g}{ch}", tag="set")
                            nc.tensor.matmul(
                                kps[:], lhsT=wkp_t[a][g][:],
                                rhs=xbT_t[:, 512 * ch: 512 * (ch + 1)],
                                start=True, stop=True)
                            nc.scalar.copy(
                                out=ktp[a][g][:, 512 * ch: 512 * (ch + 1)],
                                in_=kps[:])
                for t in range(KT):
                    vp = pset.tile([128, 512], F32, name=f"vp{t}", tag="set")
                    nc.tensor.matmul(
                        vp[:], lhsT=xbT_t[:, 128 * t: 128 * (t + 1)],
                        rhs=wv_t[:], start=True, stop=True)
                    # softmax-denominator ones column per 32-col head block
                    nc.vector.memset(
                        vp[:].rearrange("p (c x) -> p c x", x=32)[:, :, 16:32],
                        1.0)
                    nc.vector.tensor_copy(out=v_all[:, t, :], in_=vp[:])

            # ---------------- attention loops
            def attention(ai, pa):
                """8 heads as 2 quads x 2 pairs; per (pair, k-tile): 8
                tile-positioned score matmuls (2 heads x 4 col-tiles of 32
                keys, K=16 contraction) into one [128,1024] PSUM tile, one
                exp (engine per EXP_SCHED), then 2 col-tiled PV matmuls
                accumulating 4 heads into one PSUM bank per quad."""
                packed = [pp.tile([128, 512], F32R, name=f"pk{ai}{g}",
                                  tag=f"pk{ai}{g}") for g in range(2)]
                with tc.tile_pool(name=f"ebp{ai}", bufs=3) as ebp:
                    for g in range(2):
                        for half in range(2):
                            # per head: PV split into lo/hi key halves on
                            # different quadrant rows; waves [A-lo|B-hi] and
                            # [A-hi|B-lo] co-execute on disjoint rows
                            pv = [pa.tile([32, 512], F32,
                                          name=f"pv{ai}{g}{half}{i}",
                                          tag=f"pv{i}") for i in range(2)]
                            for t in range(KT):
                                sc = pa.tile([128, 1024], F32, bufs=3,
                                             name=f"sc{ai}{g}{half}{t}",
                                             tag="sc")
                                for i in range(2):
                                    r = 2 * half + i
                                    nc.tensor.matmul(
                                        sc[:, 512 * i: 512 * (i + 1)],
                                        lhsT=ktp[ai][g][
                                            32 * r: 32 * r + HD,
                                            128 * t: 128 * (t + 1)],
                                        rhs=qp[ai][g][32 * r: 32 * r + HD, :],
                                        start=True, stop=True,
                                        tile_position=(32 * r, 0),
                                        skip_group_check=True,
                                    )
                                eb = ebp.tile([128, 1024], BF16, name="eb",
                                              tag="eb")
                                eng = EXP_SCHED[t]
                                if eng == "S":
                                    nc.scalar.activation(eb[:], sc[:], AF.Exp,
                                                         scale=0.25)
                                else:
                                    e = nc.vector if eng == "V" else nc.gpsimd
                                    e.tensor_scalar(
                                        out=eb[:].bitcast(I16), in0=sc[:],
                                        scalar1=SCHRA_A, scalar2=SCHRA_B,
                                        op0=OP.mult, op1=OP.add)
                                v0 = [256 * ai + 128 * g + 32 * (2 * half + i)
                                      for i in range(2)]
                                if PV_SPLIT:
                                    for i, lo in ((0, 0), (1, 1), (0, 1), (1, 0)):
                                        nc.tensor.matmul(
                                            pv[i][:],
                                            lhsT=v_all[64 * lo: 64 * (lo + 1),
                                                       t, v0[i]: v0[i] + 32],
                                            rhs=eb[64 * lo: 64 * (lo + 1),
                                                   512 * i: 512 * (i + 1)],
                                            start=(t == 0 and i == lo),
                                            stop=(t == KT - 1 and i != lo),
                                            tile_position=(64 * lo, 0),
                                            skip_group_check=True,
                                        )
                                else:
                                    for i in range(2):
                                        nc.tensor.matmul(
                                            pv[i][:],
                                            lhsT=v_all[:, t, v0[i]: v0[i] + 32],
                                            rhs=eb[:, 512 * i: 512 * (i + 1)],
                                            start=(t == 0), stop=(t == KT - 1),
                                            skip_group_check=True,
                                        )
                            for i in range(2):
                                h4 = 2 * half + i
                                nc.vector.tensor_copy(
                                    out=packed[g][32 * h4: 32 * h4 + 32, :],
                                    in_=pv[i][:])
                # reciprocal of the packed accumulators (only the denominator
                # rows 32c+16 are consumed downstream); runs on DVE overlapped
                # with the next phase instead of serializing the tail
                rbs = []
                for g in range(2):
                    rb = pp.tile([128, 512], F32R, name=f"rb{ai}{g}",
                                 tag=f"rb{ai}{g}")
                    with nc.allow_low_precision(reason="f32r is bit-identical f32"):
                        nc.vector.reciprocal(out=rb[:], in_=packed[g][:])
                    rbs.append(rb)
                return packed, rbs

            def normalize_and_project(ai, pvrb, wp_t, res_prev, b_t, pa):
                """softmax-normalize the packed PV accumulators (reciprocals
                precomputed) and apply the merge projection."""
                pv, rbs = pvrb
                sa_n = []
                for grp in range(2):
                    sbc = pa.tile([128, 512], F32, name=f"sbc{ai}{grp}",
                                  tag=f"ps{grp}")
                    nc.tensor.matmul(sbc[:], lhsT=sel_t[:], rhs=rbs[grp][:],
                                     start=True, stop=True)
                    sn = pp.tile([128, 512], F32R, name=f"sn{ai}{grp}",
                                 tag=f"sn{grp}")
                    nc.vector.tensor_mul(sn[:], pv[grp][:], sbc[:])
                    sa_n.append(sn)
                rp = pa.tile([128, 512], F32, name=f"rp{ai}", tag="ps0")
                for grp in range(2):
                    nc.tensor.matmul(rp[:], lhsT=wp_t[grp][:],
                                     rhs=sa_n[grp][:],
                                     start=(grp == 0), stop=(grp == 1))
                rT = sbuf(f"r{ai}T", (128, 512), F32R)
                nc.vector.tensor_add(rT[:], rp[:], res_prev[:])
                nc.vector.tensor_scalar_add(rT[:], rT[:], b_t[:, 0:1])
                return rT

            with tc.tile_pool(name="pattn", bufs=1, space="PSUM") as pa:
                pv_s = attention(0, pa)
                pv_c = attention(1, pa)
            with tc.tile_pool(name="pnorm", bufs=1, space="PSUM") as pa:
                r1T = normalize_and_project(0, pv_s, w1p_t, xqT_t, b1_t, pa)
                r2T = normalize_and_project(1, pv_c, w2p_t, r1T, b2_t, pa)

            # ---------------- layernorm (x - m) / var, var unbiased
            with tc.tile_pool(name="ptail", bufs=1, space="PSUM") as pt:
                sq = sbuf("sq", (128, 512), F32R)
                nc.vector.tensor_mul(sq[:], r2T[:], r2T[:])
                mp = pt.tile([1, 512], F32, name="mp", tag="st0")
                nc.tensor.matmul(mp[:], lhsT=onec_t[:], rhs=r2T[:],
                                 start=True, stop=True)
                sp = pt.tile([1, 512], F32, name="sp", tag="st1")
                nc.tensor.matmul(sp[:], lhsT=onec_t[:], rhs=sq[:],
                                 start=True, stop=True)
                msb = sbuf("msb", (1, 512))
                nc.vector.tensor_copy(out=msb[:], in_=mp[:])
                t0 = sbuf("t0", (1, 512))
                # t0 = (mp * -1/128) * mp + sp = sum((x-m)^2)
                nc.vector.tensor_scalar(out=t0[:], in0=msb[:],
                                        scalar1=-1.0 / 128, scalar2=None,
                                        op0=OP.mult)
                nc.vector.tensor_mul(t0[:], t0[:], msb[:])
                nc.vector.tensor_add(t0[:], t0[:], sp[:])
                asb_f = sbuf("asb_f", (1, 512))
                nc.vector.reciprocal(out=asb_f[:], in_=t0[:])
                asb = sbuf("asb", (1, 512), F32R)
                nc.vector.tensor_scalar_mul(asb[:], asb_f[:], 127.0)  # 1/var
                bsb = sbuf("bsb", (1, 512), F32R)
                nc.vector.tensor_mul(bsb[:], msb[:], asb[:])
                nc.vector.tensor_scalar_mul(bsb[:], bsb[:], -1.0 / 128)
                abc = pt.tile([128, 512], F32, name="abc", tag="bc0")
                nc.tensor.matmul(abc[:], lhsT=oner_t[:], rhs=asb[:],
                                 start=True, stop=True)
                bbc = pt.tile([128, 512], F32, name="bbc", tag="bc1")
                nc.tensor.matmul(bbc[:], lhsT=oner_t[:], rhs=bsb[:],
                                 start=True, stop=True)
                lnT = sbuf("lnT", (128, 512), F32R)
                nc.vector.tensor_mul(lnT[:], r2T[:], abc[:])
                nc.vector.tensor_add(lnT[:], lnT[:], bbc[:])

                # ---------------- FFN
                h_sb = []
                for j in range(4):
                    hp = pt.tile([128, 512], F32, name=f"hp{j}", tag=f"hp{j % 2}")
                    nc.tensor.matmul(hp[:],
                                     lhsT=w3_t[:, 128 * j: 128 * (j + 1)],
                                     rhs=lnT[:], start=True, stop=True)
                    hs = sbuf(f"hs{j}", (128, 512), F32R)
                    nc.vector.tensor_scalar(
                        out=hs[:], in0=hp[:], scalar1=b3_t[:, j: j + 1],
                        scalar2=0.0, op0=OP.add, op1=OP.max,
                    )
                    h_sb.append(hs)
                op_ = pt.tile([128, 512], F32, name="op", tag="bc0")
                for j in range(4):
                    nc.tensor.matmul(op_[:],
                                     lhsT=w4_t[:, 128 * j: 128 * (j + 1)],
                                     rhs=h_sb[j][:],
                                     start=(j == 0), stop=(j == 3),
                                     skip_group_check=True)
                oT = sbuf("oT", (128, 512))
                nc.vector.tensor_add(oT[:], op_[:], r2T[:])
                nc.vector.tensor_scalar_add(oT[:], oT[:], b4_t[:, 0:1])
                nc.sync.dma_start(out=y[:], in_=oT[:])

    return nc


_CACHED = {}


def _get_nc():
    if "nc" not in _CACHED:
        _CACHED["nc"] = _split_multiwaits(build_nc())
    return _CACHED["nc"]


def _host_inputs(x_tgt, enc_out, self_wq, self_wk, self_wv, cross_wq, cross_wk,
                 cross_wv, w1, b1, w2, b2, w3, b3, w4, b4):
    shared = {
        "wkp00": _wk_quad(self_wk, 0), "wkp01": _wk_quad(self_wk, 1),
        "wkp10": _wk_quad(cross_wk, 0), "wkp11": _wk_quad(cross_wk, 1),
        "wqp00": _wk_quad(self_wq, 0), "wqp01": _wk_quad(self_wq, 1),
        "wqp10": _wk_quad(cross_wq, 0), "wqp11": _wk_quad(cross_wq, 1),
        "wv_st": np.concatenate(
            [_pack32_cols(self_wv, 0), _pack32_cols(self_wv, 1),
             _pack32_cols(cross_wv, 0), _pack32_cols(cross_wv, 1)], axis=1
        ),
        "w1p0": _pack_w12(w1, 0), "w1p1": _pack_w12(w1, 1),
        "w2p0": _pack_w12(w2, 0), "w2p1": _pack_w12(w2, 1),
        "w3": w3,
        "w4r": np.ascontiguousarray(
            w4.reshape(4, 128, 128).transpose(1, 0, 2).reshape(128, 512)
        ),
        "selt": _sel_matrix(),
        "ones_col": np.ones((128, 1), np.float32),
        "ones_row": np.ones((1, 128), np.float32),
        "b1t": b1.reshape(128, 1),
        "b2t": b2.reshape(128, 1),
        "b3t": np.ascontiguousarray(b3.reshape(4, 128).T),
        "b4t": b4.reshape(128, 1),
    }
    bf16 = {"wkp00", "wkp01", "wkp10", "wkp11", "wv_st"}
    shared = {k: np.ascontiguousarray(
        v, dtype=(ml_dtypes.bfloat16 if k in bf16 else np.float32))
              for k, v in shared.items()}
    in_maps = []
    for c in range(NCORES):
        b, qb = divmod(c, 4)
        q0 = qb * QC
        im = dict(shared)
        im["xbT"] = np.ascontiguousarray(x_tgt[b].T, dtype=ml_dtypes.bfloat16)
        im["xqT"] = np.ascontiguousarray(x_tgt[b, q0: q0 + QC].T)
        im["eoT"] = np.ascontiguousarray(enc_out[b, q0: q0 + QC].T)
        in_maps.append(im)
    return in_maps


def _unshuf(y):
    """device y [128, 512] is out^T for this core's query slice."""
    return np.ascontiguousarray(y.T)


def run_on_device(in_maps, **kw):
    nc = _get_nc()
    return run_bass_kernel_spmd(nc, in_maps, list(range(NCORES)), **kw)


def kernel(x_tgt, enc_out, self_wq, self_wk, self_wv, cross_wq, cross_wk,
           cross_wv, w1, b1, w2, b2, w3, b3, w4, b4, mask_src=None,
           mask_tgt=None, **_unused):
    args = [x_tgt, enc_out, self_wq, self_wk, self_wv, cross_wq, cross_wk,
            cross_wv, w1, b1, w2, b2, w3, b3, w4, b4]
    args = [np.asarray(a, dtype=np.float32) for a in args]
    in_maps = _host_inputs(*args)
    res = run_on_device(in_maps)
    out = np.empty((B, S, D), np.float32)
    for c in range(NCORES):
        b, qb = divmod(c, 4)
        out[b, qb * QC: (qb + 1) * QC] = _unshuf(res.results[c]["y"])
    return out
